# revision 14
# baseline (speedup 1.0000x reference)
"""Trainium2 Bass kernel for nn_ClauseInferModule (gnn_message_passing).

out[c, b, g] = sum_s prod_l x[b, I[c, g, s, l]],  B=16 G=16384 C=8 S=8 L=4.

Sharding: clause-per-core (C == n_cores == 8). Per core:
  - x arrives host-replicated as [128, G] f32 (partition p = batch p % 16,
    copied for the 8 GPSIMD core groups) and is staged with full-width
    column-split DMAs on the SP + Activation HWDGE queues + Pool SWDGE,
  - GPSIMD ap_gather pulls x[b, idx] for 16 b at once (idx shared across the
    16 partitions of a Q7 core group); each of the 8 groups processes its own
    2048-atom slice of the clause for one substitution s per call,
  - VectorE multiplies the L=4 gathered blocks (step-1 fp32 tensor_tensor)
    and accumulates over the S=8 substitutions,
  - the [128, 2048] accumulator is written wide to HBM; the host
    de-interleaves [16k+b, w] -> [b, k*2048+w].

The int64 index tensor is converted to the int16 "wrapped in 16 partitions"
ap_gather layout on the host (a pure dtype/layout transform).

Structure: raw bacc Block threads (no TileContext) with a 4-semaphore
protocol - TileContext's epilogue alone cost ~10us of semaphore-drain
cascade, and its per-call dependency checks ~0.9us per gather. The last
substitution is gathered in four 2048-idx quarter-calls so its vector
chain + output DMA overlap the remaining quarters' gathers.

Perf notes (HW-measured): the substitutions' ap_gather calls dominate
(~222us per 65536-idx full call, 27.1ns/idx/Q7-core = 3.39ns/idx
aggregate); the floor is the ~102-cycle reset_reads RD_CMD serving 4
indices in the ap_gather ucode (cayman ReadOverlap=0 - RD_CMDs do not
pipeline), so no caller-side change can beat it. Vector work and idx
staging hide under the gathers.

Alternatives measured on HW, all dead ends:
  - dma_gather (InstDMAGatherAnt, HBM rows, f32 elem 64): 4.6us GPSIMD-busy
    per 1024-idx call on 4 SWDGE queues = 3.33ns/idx aggregate - no better
    than ap_gather (desc-gen is Q7-serial on the same engine, ~2.3cyc/idx).
    Calls >1024 idx (>65 descs/lane) wedge this runtime, and a failed call
    poisons the device for later SWDGE-gather kernels until a kernel with a
    different GPSIMD library runs ("cleansing").
  - indirect_copy (InstIndirectCopy): rejected by neuronxcc walrus
    ("ISA check failed", NCC_IXCG864).
  - On-device x fan-out ([16,G] HBM + 7 SBUF->SBUF copies): slower than
    host-replicated [128,G] staging (~296-346GB/s, already HBM-bound).
"""
import os
import sys
import numpy as np

sys.path.insert(0, "/opt/trn_rl_repo")

import concourse.bacc as bacc
import concourse.bass as bass
from concourse import library_config, mybir
from concourse.bass_utils import run_bass_kernel_spmd

B, G = 16, 16384
C, S, L = 8, 8, 4
NIDX = 8192          # gathers per core group per full ap_gather call
GCHUNK = G // 8      # 2048 target atoms per core group
NQ = 4               # last substitution split into NQ quarter-calls
QIDX = NIDX // NQ    # idx per quarter-call
QW = GCHUNK // NQ    # g-window per quarter-call

_compiled = None
last_exec_time_ns = None


def _ensure_ntff_hook():
    """Register the axon NTFF profile hook if the antenv stub lacks it."""
    import types
    try:
        from antenv.axon_hooks import get_axon_ntff_profile_hook  # noqa: F401
        return
    except ImportError:
        pass
    try:
        import antenv
        from trn_agent_boot.trn_boot import _ntff_profile_via_ctypes
        mod = types.ModuleType("antenv.axon_hooks")
        _hook = [None]
        mod.set_axon_ntff_profile_hook = lambda h: _hook.__setitem__(0, h)
        mod.get_axon_ntff_profile_hook = lambda: _hook[0]
        sys.modules["antenv.axon_hooks"] = mod
        antenv.axon_hooks = mod
        hook = _ntff_profile_via_ctypes("/opt/axon/libaxon_pjrt.so")
        if hook is not None:
            mod.set_axon_ntff_profile_hook(hook)
    except Exception:
        pass


def _build():
    # detect_race_conditions=False: the CoreSim race detector has no
    # same-engine program-order model for raw (non-Tile) kernels and flags
    # in-order DVE chains; cross-engine ordering here is explicit via sems.
    nc = bacc.Bacc("TRN2", target_bir_lowering=False, debug=False,
                   detect_race_conditions=False)
    x_d = nc.dram_tensor("x", [128, G], mybir.dt.float32, kind="ExternalInput")
    idx_d = nc.dram_tensor("idx", [128, S * (NIDX // 16)], mybir.dt.int16,
                           kind="ExternalInput")
    # wide output slab; host de-interleaves [16k+b, w] -> [b, k*2048+w]
    out_d = nc.dram_tensor("out", [128, GCHUNK], mybir.dt.float32,
                           kind="ExternalOutput")

    icols = S * (NIDX // 16)
    # calls: 7 full substitutions + NQ quarter-calls for s = S-1.
    # call j -> (idx column offset, idx count, g-window lo for s7 quarters)
    calls = [(s * (NIDX // 16), NIDX, None) for s in range(S - 1)]
    base = (S - 1) * (NIDX // 16)
    calls += [(base + q * (QIDX // 16), QIDX, q * QW) for q in range(NQ)]
    ncalls = len(calls)

    with (
        nc.Block() as block,
        nc.sbuf_tensor("xt", [128, G], mybir.dt.float32) as x_tile,
        nc.sbuf_tensor("it", [128, icols], mybir.dt.int16) as itall,
        nc.sbuf_tensor("g0", [128, NIDX], mybir.dt.float32) as g0,
        nc.sbuf_tensor("g1", [128, NIDX], mybir.dt.float32) as g1,
        nc.sbuf_tensor("acc", [128, GCHUNK], mybir.dt.float32) as acc,
        nc.sbuf_tensor("tm1", [128, GCHUNK], mybir.dt.float32) as tm1,
        nc.sbuf_tensor("tm2", [128, GCHUNK], mybir.dt.float32) as tm2,
        nc.sbuf_tensor("tm3", [128, GCHUNK], mybir.dt.float32) as tm3,
        nc.semaphore("stage") as sem_stage,
        nc.semaphore("stageg") as sem_stageg,
        nc.semaphore("g") as sem_g,
        nc.semaphore("v") as sem_v,
        nc.semaphore("o") as sem_o,
    ):
        gt = (g0, g1)

        @block.sync
        def _(sync: bass.BassEngine):
            a = G // 3
            sync.dma_start(itall[:, :icols // 2],
                           idx_d[:, :icols // 2]).then_inc(sem_stage, 16)
            sync.dma_start(x_tile[:, :a], x_d[:, :a]).then_inc(sem_stage, 16)
            # out halves: g-windows 0..1023 done after quarter chains 0-1
            # (sem_v = 7 full + 2 quarters = 9), 1024..2047 after all 11.
            sync.wait_ge(sem_v, S - 1 + 2)
            sync.dma_start(out_d[:, :GCHUNK // 2],
                           acc[:, :GCHUNK // 2]).then_inc(sem_o, 16)
            sync.wait_ge(sem_o, 32)

        @block.scalar
        def _(scalar: bass.BassEngine):
            a, b = G // 3, 2 * (G // 3)
            scalar.dma_start(itall[:, icols // 2:],
                             idx_d[:, icols // 2:]).then_inc(sem_stage, 16)
            scalar.dma_start(x_tile[:, a:b],
                             x_d[:, a:b]).then_inc(sem_stage, 16)
            scalar.wait_ge(sem_v, S - 1 + NQ)
            scalar.dma_start(out_d[:, GCHUNK // 2:],
                             acc[:, GCHUNK // 2:]).then_inc(sem_o, 16)
            scalar.wait_ge(sem_o, 32)

        @block.gpsimd
        def _(gpsimd: bass.BassGpSimd):
            b = 2 * (G // 3)
            gpsimd.dma_start(x_tile[:, b:],
                             x_d[:, b:]).then_inc(sem_stageg, 16)
            gpsimd.load_library(library_config.ap_gather)
            gpsimd.wait_ge(sem_stage, 4 * 16)
            gpsimd.wait_ge(sem_stageg, 16)
            for j, (coff, nidx, _wlo) in enumerate(calls):
                if j >= 2:
                    # WAR: call j reuses gt[j%2]; vector chain j-2 must be
                    # done with it (sem_v counts completed chains).
                    gpsimd.wait_ge(sem_v, j - 1)
                g = gt[j % 2]
                it = itall[:, coff:coff + nidx // 16]
                gpsimd.ap_gather(g[:, :nidx], x_tile[:, :], it[:, :],
                                 channels=128, num_elems=G, d=1,
                                 num_idxs=nidx).then_inc(sem_g, 1)

        @block.vector
        def _(vector: bass.BassEngine):
            for j, (coff, nidx, wlo) in enumerate(calls):
                g = gt[j % 2]
                w = nidx // L  # g-window width of this call
                vector.wait_ge(sem_g, j + 1)

                def A(l):
                    return g[:, l * w:(l + 1) * w]

                vector.tensor_mul(tm1[:, :w], A(0), A(1))
                vector.tensor_mul(tm2[:, :w], A(2), A(3))
                if j == 0:
                    vector.tensor_mul(acc[:, :w], tm1[:, :w],
                                      tm2[:, :w]).then_inc(sem_v, 1)
                else:
                    lo = wlo if wlo is not None else 0
                    vector.tensor_mul(tm3[:, :w], tm1[:, :w], tm2[:, :w])
                    vector.tensor_add(acc[:, lo:lo + w], acc[:, lo:lo + w],
                                      tm3[:, :w]).then_inc(sem_v, 1)

    nc.compile()
    return nc


def _prep_idx(I: np.ndarray) -> np.ndarray:
    """[C, G, S, L] int64 -> [C, 128, S*512] int16 wrapped ap_gather feed.

    Call j covers substitution s over a g-window [wlo, whi) of each core
    group's 2048-atom slice: stream position i = l*(whi-wlo) + (w-wlo) holds
    I[c, k*2048 + w, s, l]; ap_gather reads position i of group k from
    it[16*k + i%16, i//16]. Calls: s=0..6 full windows, s=7 in quarters.
    """
    T = I.astype(np.int16).reshape(C, 8, GCHUNK, S, L)     # [c,k,w,s,l]
    calls = [(s, 0, GCHUNK) for s in range(S - 1)]
    calls += [(S - 1, q * QW, (q + 1) * QW) for q in range(NQ)]
    blocks = []
    for s, wlo, whi in calls:
        wn = whi - wlo
        st = T[:, :, wlo:whi, s, :]                        # [c,k,w,l]
        st = st.transpose(0, 1, 3, 2).reshape(C, 8, L * wn)  # i = l*wn + w
        wr = st.reshape(C, 8, (L * wn) // 16, 16)          # [c,k,col,pp]
        blocks.append(wr.transpose(0, 1, 3, 2))            # [c,k,pp,col]
    W = np.concatenate(blocks, axis=3)                     # [c,k,pp,allcol]
    return np.ascontiguousarray(W).reshape(C, 128, S * (NIDX // 16))


def kernel(x: np.ndarray, I: np.ndarray) -> np.ndarray:
    global _compiled, last_exec_time_ns
    if _compiled is None:
        _compiled = _build()
    nc = _compiled

    x = np.ascontiguousarray(np.asarray(x), dtype=np.float32)
    xrep = np.ascontiguousarray(np.tile(x, (8, 1)))  # [128, G], p = b%16
    idx_feed = _prep_idx(np.asarray(I))

    in_maps = [{"x": xrep, "idx": idx_feed[c]} for c in range(C)]
    kwargs = {}
    if os.environ.get("KERNEL_TRACE") == "1":
        _ensure_ntff_hook()
        kwargs = {"trace": True, "trace_cores": list(range(C))}
    res = run_bass_kernel_spmd(nc, in_maps, core_ids=list(range(C)), **kwargs)
    last_exec_time_ns = res.exec_time_ns
    # wide slab [16k+b, w] -> [b, k*2048+w]
    out = np.stack(
        [res.results[c]["out"].reshape(8, B, GCHUNK).transpose(1, 0, 2)
         .reshape(B, G) for c in range(C)], axis=0)
    return np.ascontiguousarray(out, dtype=np.float32)


if __name__ == "__main__":
    rng = np.random.default_rng(0)
    x = rng.random((B, G), dtype=np.float32)
    I = rng.integers(0, G, size=(C, G, S, L)).astype(np.int64)
    out = kernel(x=x, I=I)
    gathered = x[:, I]
    expect = np.moveaxis(np.sum(np.prod(gathered, axis=-1), axis=-1), 0, 1)
    err = np.abs(out - expect).max() / np.abs(expect).max()
    print("max rel err:", err)


# revision 15
# speedup vs baseline: 1.0018x; 1.0018x over previous
"""Trainium2 Bass kernel for nn_ClauseInferModule (gnn_message_passing).

out[c, b, g] = sum_s prod_l x[b, I[c, g, s, l]],  B=16 G=16384 C=8 S=8 L=4.

Sharding: clause-per-core (C == n_cores == 8). Per core:
  - x arrives host-replicated as [128, G] f32 (partition p = batch p % 16,
    copied for the 8 GPSIMD core groups) and is staged with full-width
    column-split DMAs on the SP + Activation HWDGE queues + Pool SWDGE,
  - GPSIMD ap_gather pulls x[b, idx] for 16 b at once (idx shared across the
    16 partitions of a Q7 core group); each of the 8 groups processes its own
    2048-atom slice of the clause for one substitution s per call,
  - VectorE multiplies the L=4 gathered blocks (step-1 fp32 tensor_tensor)
    and accumulates over the S=8 substitutions,
  - the [128, 2048] accumulator is written wide to HBM; the host
    de-interleaves [16k+b, w] -> [b, k*2048+w].

The int64 index tensor is converted to the int16 "wrapped in 16 partitions"
ap_gather layout on the host (a pure dtype/layout transform).

Structure: raw bacc Block threads (no TileContext) with a 4-semaphore
protocol - TileContext's epilogue alone cost ~10us of semaphore-drain
cascade, and its per-call dependency checks ~0.9us per gather. The last
substitution is gathered in four 2048-idx quarter-calls so its vector
chain + output DMA overlap the remaining quarters' gathers.

Perf notes (HW-measured): the substitutions' ap_gather calls dominate
(~222us per 65536-idx full call, 27.1ns/idx/Q7-core = 3.39ns/idx
aggregate); the floor is the ~102-cycle reset_reads RD_CMD serving 4
indices in the ap_gather ucode (cayman ReadOverlap=0 - RD_CMDs do not
pipeline), so no caller-side change can beat it. Vector work and idx
staging hide under the gathers.

Alternatives measured on HW, all dead ends:
  - dma_gather (InstDMAGatherAnt, HBM rows, f32 elem 64): 4.6us GPSIMD-busy
    per 1024-idx call on 4 SWDGE queues = 3.33ns/idx aggregate - no better
    than ap_gather (desc-gen is Q7-serial on the same engine, ~2.3cyc/idx).
    Calls >1024 idx (>65 descs/lane) wedge this runtime, and a failed call
    poisons the device for later SWDGE-gather kernels until a kernel with a
    different GPSIMD library runs ("cleansing").
  - indirect_copy (InstIndirectCopy): rejected by neuronxcc walrus
    ("ISA check failed", NCC_IXCG864).
  - On-device x fan-out ([16,G] HBM + 7 SBUF->SBUF copies): slower than
    host-replicated [128,G] staging (~296-346GB/s, already HBM-bound).
"""
import os
import sys
import numpy as np

sys.path.insert(0, "/opt/trn_rl_repo")

import concourse.bacc as bacc
import concourse.bass as bass
from concourse import library_config, mybir
from concourse.bass_utils import run_bass_kernel_spmd

B, G = 16, 16384
C, S, L = 8, 8, 4
NIDX = 8192          # gathers per core group per full ap_gather call
GCHUNK = G // 8      # 2048 target atoms per core group
NQ = 2               # last substitution split into NQ sub-calls (halves:
                     # each extra ap_gather call costs ~2.6us fixed, so a
                     # finer split loses more than the shorter tail saves)
QIDX = NIDX // NQ    # idx per quarter-call
QW = GCHUNK // NQ    # g-window per quarter-call

_compiled = None
last_exec_time_ns = None


def _ensure_ntff_hook():
    """Register the axon NTFF profile hook if the antenv stub lacks it."""
    import types
    try:
        from antenv.axon_hooks import get_axon_ntff_profile_hook  # noqa: F401
        return
    except ImportError:
        pass
    try:
        import antenv
        from trn_agent_boot.trn_boot import _ntff_profile_via_ctypes
        mod = types.ModuleType("antenv.axon_hooks")
        _hook = [None]
        mod.set_axon_ntff_profile_hook = lambda h: _hook.__setitem__(0, h)
        mod.get_axon_ntff_profile_hook = lambda: _hook[0]
        sys.modules["antenv.axon_hooks"] = mod
        antenv.axon_hooks = mod
        hook = _ntff_profile_via_ctypes("/opt/axon/libaxon_pjrt.so")
        if hook is not None:
            mod.set_axon_ntff_profile_hook(hook)
    except Exception:
        pass


def _build():
    # detect_race_conditions=False: the CoreSim race detector has no
    # same-engine program-order model for raw (non-Tile) kernels and flags
    # in-order DVE chains; cross-engine ordering here is explicit via sems.
    nc = bacc.Bacc("TRN2", target_bir_lowering=False, debug=False,
                   detect_race_conditions=False)
    x_d = nc.dram_tensor("x", [128, G], mybir.dt.float32, kind="ExternalInput")
    idx_d = nc.dram_tensor("idx", [128, S * (NIDX // 16)], mybir.dt.int16,
                           kind="ExternalInput")
    # wide output slab; host de-interleaves [16k+b, w] -> [b, k*2048+w]
    out_d = nc.dram_tensor("out", [128, GCHUNK], mybir.dt.float32,
                           kind="ExternalOutput")

    icols = S * (NIDX // 16)
    # calls: 7 full substitutions + NQ quarter-calls for s = S-1.
    # call j -> (idx column offset, idx count, g-window lo for s7 quarters)
    calls = [(s * (NIDX // 16), NIDX, None) for s in range(S - 1)]
    base = (S - 1) * (NIDX // 16)
    calls += [(base + q * (QIDX // 16), QIDX, q * QW) for q in range(NQ)]
    ncalls = len(calls)

    with (
        nc.Block() as block,
        nc.sbuf_tensor("xt", [128, G], mybir.dt.float32) as x_tile,
        nc.sbuf_tensor("it", [128, icols], mybir.dt.int16) as itall,
        nc.sbuf_tensor("g0", [128, NIDX], mybir.dt.float32) as g0,
        nc.sbuf_tensor("g1", [128, NIDX], mybir.dt.float32) as g1,
        nc.sbuf_tensor("acc", [128, GCHUNK], mybir.dt.float32) as acc,
        nc.sbuf_tensor("tm1", [128, GCHUNK], mybir.dt.float32) as tm1,
        nc.sbuf_tensor("tm2", [128, GCHUNK], mybir.dt.float32) as tm2,
        nc.sbuf_tensor("tm3", [128, GCHUNK], mybir.dt.float32) as tm3,
        nc.semaphore("stage") as sem_stage,
        nc.semaphore("stageg") as sem_stageg,
        nc.semaphore("g") as sem_g,
        nc.semaphore("v") as sem_v,
        nc.semaphore("o") as sem_o,
    ):
        gt = (g0, g1)

        @block.sync
        def _(sync: bass.BassEngine):
            a = G // 3
            # x bulk first: a small DMA at the queue head would delay the
            # bulk transfer the first gather waits on (measured +6us).
            sync.dma_start(x_tile[:, :a], x_d[:, :a]).then_inc(sem_stage, 16)
            sync.dma_start(itall[:, :icols // 2],
                           idx_d[:, :icols // 2]).then_inc(sem_stage, 16)
            # half 0 of out (g-windows 0..1023) is final after sub-call
            # chain S-1+1; half 1 after all S-1+NQ chains.
            sync.wait_ge(sem_v, S - 1 + 1)
            sync.dma_start(out_d[:, :GCHUNK // 2],
                           acc[:, :GCHUNK // 2]).then_inc(sem_o, 16)
            sync.wait_ge(sem_o, 32)

        @block.scalar
        def _(scalar: bass.BassEngine):
            a, b = G // 3, 2 * (G // 3)
            scalar.dma_start(x_tile[:, a:b],
                             x_d[:, a:b]).then_inc(sem_stage, 16)
            scalar.dma_start(itall[:, icols // 2:],
                             idx_d[:, icols // 2:]).then_inc(sem_stage, 16)
            scalar.wait_ge(sem_v, S - 1 + NQ)
            scalar.dma_start(out_d[:, GCHUNK // 2:],
                             acc[:, GCHUNK // 2:]).then_inc(sem_o, 16)
            scalar.wait_ge(sem_o, 32)

        @block.gpsimd
        def _(gpsimd: bass.BassGpSimd):
            b = 2 * (G // 3)
            gpsimd.dma_start(x_tile[:, b:],
                             x_d[:, b:]).then_inc(sem_stageg, 16)
            gpsimd.load_library(library_config.ap_gather)
            gpsimd.wait_ge(sem_stage, 4 * 16)
            gpsimd.wait_ge(sem_stageg, 16)
            for j, (coff, nidx, _wlo) in enumerate(calls):
                if j >= 2:
                    # WAR: call j reuses gt[j%2]; vector chain j-2 must be
                    # done with it (sem_v counts completed chains).
                    gpsimd.wait_ge(sem_v, j - 1)
                g = gt[j % 2]
                it = itall[:, coff:coff + nidx // 16]
                gpsimd.ap_gather(g[:, :nidx], x_tile[:, :], it[:, :],
                                 channels=128, num_elems=G, d=1,
                                 num_idxs=nidx).then_inc(sem_g, 1)

        @block.vector
        def _(vector: bass.BassEngine):
            for j, (coff, nidx, wlo) in enumerate(calls):
                g = gt[j % 2]
                w = nidx // L  # g-window width of this call
                vector.wait_ge(sem_g, j + 1)

                def A(l):
                    return g[:, l * w:(l + 1) * w]

                vector.tensor_mul(tm1[:, :w], A(0), A(1))
                vector.tensor_mul(tm2[:, :w], A(2), A(3))
                if j == 0:
                    vector.tensor_mul(acc[:, :w], tm1[:, :w],
                                      tm2[:, :w]).then_inc(sem_v, 1)
                else:
                    lo = wlo if wlo is not None else 0
                    vector.tensor_mul(tm3[:, :w], tm1[:, :w], tm2[:, :w])
                    vector.tensor_add(acc[:, lo:lo + w], acc[:, lo:lo + w],
                                      tm3[:, :w]).then_inc(sem_v, 1)

    nc.compile()
    return nc


def _prep_idx(I: np.ndarray) -> np.ndarray:
    """[C, G, S, L] int64 -> [C, 128, S*512] int16 wrapped ap_gather feed.

    Call j covers substitution s over a g-window [wlo, whi) of each core
    group's 2048-atom slice: stream position i = l*(whi-wlo) + (w-wlo) holds
    I[c, k*2048 + w, s, l]; ap_gather reads position i of group k from
    it[16*k + i%16, i//16]. Calls: s=0..6 full windows, s=7 in quarters.
    """
    T = I.astype(np.int16).reshape(C, 8, GCHUNK, S, L)     # [c,k,w,s,l]
    calls = [(s, 0, GCHUNK) for s in range(S - 1)]
    calls += [(S - 1, q * QW, (q + 1) * QW) for q in range(NQ)]
    blocks = []
    for s, wlo, whi in calls:
        wn = whi - wlo
        st = T[:, :, wlo:whi, s, :]                        # [c,k,w,l]
        st = st.transpose(0, 1, 3, 2).reshape(C, 8, L * wn)  # i = l*wn + w
        wr = st.reshape(C, 8, (L * wn) // 16, 16)          # [c,k,col,pp]
        blocks.append(wr.transpose(0, 1, 3, 2))            # [c,k,pp,col]
    W = np.concatenate(blocks, axis=3)                     # [c,k,pp,allcol]
    return np.ascontiguousarray(W).reshape(C, 128, S * (NIDX // 16))


def kernel(x: np.ndarray, I: np.ndarray) -> np.ndarray:
    global _compiled, last_exec_time_ns
    if _compiled is None:
        _compiled = _build()
    nc = _compiled

    x = np.ascontiguousarray(np.asarray(x), dtype=np.float32)
    xrep = np.ascontiguousarray(np.tile(x, (8, 1)))  # [128, G], p = b%16
    idx_feed = _prep_idx(np.asarray(I))

    in_maps = [{"x": xrep, "idx": idx_feed[c]} for c in range(C)]
    kwargs = {}
    if os.environ.get("KERNEL_TRACE") == "1":
        _ensure_ntff_hook()
        kwargs = {"trace": True, "trace_cores": list(range(C))}
    res = run_bass_kernel_spmd(nc, in_maps, core_ids=list(range(C)), **kwargs)
    last_exec_time_ns = res.exec_time_ns
    # wide slab [16k+b, w] -> [b, k*2048+w]
    out = np.stack(
        [res.results[c]["out"].reshape(8, B, GCHUNK).transpose(1, 0, 2)
         .reshape(B, G) for c in range(C)], axis=0)
    return np.ascontiguousarray(out, dtype=np.float32)


if __name__ == "__main__":
    rng = np.random.default_rng(0)
    x = rng.random((B, G), dtype=np.float32)
    I = rng.integers(0, G, size=(C, G, S, L)).astype(np.int64)
    out = kernel(x=x, I=I)
    gathered = x[:, I]
    expect = np.moveaxis(np.sum(np.prod(gathered, axis=-1), axis=-1), 0, 1)
    err = np.abs(out - expect).max() / np.abs(expect).max()
    print("max rel err:", err)


# revision 16
# speedup vs baseline: 1.0039x; 1.0020x over previous
"""Trainium2 Bass kernel for nn_ClauseInferModule (gnn_message_passing).

out[c, b, g] = sum_s prod_l x[b, I[c, g, s, l]],  B=16 G=16384 C=8 S=8 L=4.

Sharding: clause-per-core (C == n_cores == 8). Per core:
  - x arrives host-replicated as [128, G] f32 (partition p = batch p % 16,
    copied for the 8 GPSIMD core groups) and is staged with full-width
    column-split DMAs on the SP + Activation HWDGE queues + Pool SWDGE,
  - GPSIMD ap_gather pulls x[b, idx] for 16 b at once (idx shared across the
    16 partitions of a Q7 core group); each of the 8 groups processes its own
    2048-atom slice of the clause for one substitution s per call,
  - VectorE multiplies the L=4 gathered blocks (step-1 fp32 tensor_tensor)
    and accumulates over the S=8 substitutions,
  - the [128, 2048] accumulator is written wide to HBM; the host
    de-interleaves [16k+b, w] -> [b, k*2048+w].

The int64 index tensor is converted to the int16 "wrapped in 16 partitions"
ap_gather layout on the host (a pure dtype/layout transform).

Structure: raw bacc Block threads (no TileContext) with a 4-semaphore
protocol - TileContext's epilogue alone cost ~10us of semaphore-drain
cascade, and its per-call dependency checks ~0.9us per gather. The last
substitution is gathered in four 2048-idx quarter-calls so its vector
chain + output DMA overlap the remaining quarters' gathers.

Perf notes (HW-measured): the substitutions' ap_gather calls dominate
(~222us per 65536-idx full call, 27.1ns/idx/Q7-core = 3.39ns/idx
aggregate); the floor is the ~102-cycle reset_reads RD_CMD serving 4
indices in the ap_gather ucode (cayman ReadOverlap=0 - RD_CMDs do not
pipeline), so no caller-side change can beat it. Vector work and idx
staging hide under the gathers.

Alternatives measured on HW, all dead ends:
  - dma_gather (InstDMAGatherAnt, HBM rows, f32 elem 64): 4.6us GPSIMD-busy
    per 1024-idx call on 4 SWDGE queues = 3.33ns/idx aggregate - no better
    than ap_gather (desc-gen is Q7-serial on the same engine, ~2.3cyc/idx).
    Calls >1024 idx (>65 descs/lane) wedge this runtime, and a failed call
    poisons the device for later SWDGE-gather kernels until a kernel with a
    different GPSIMD library runs ("cleansing").
  - indirect_copy (InstIndirectCopy): rejected by neuronxcc walrus
    ("ISA check failed", NCC_IXCG864).
  - On-device x fan-out ([16,G] HBM + 7 SBUF->SBUF copies): slower than
    host-replicated [128,G] staging (~296-346GB/s, already HBM-bound).
"""
import os
import sys
import numpy as np

sys.path.insert(0, "/opt/trn_rl_repo")

import concourse.bacc as bacc
import concourse.bass as bass
from concourse import library_config, mybir
from concourse.bass_utils import run_bass_kernel_spmd

B, G = 16, 16384
C, S, L = 8, 8, 4
NIDX = 8192          # gathers per core group per full ap_gather call
GCHUNK = G // 8      # 2048 target atoms per core group
NQ = 2               # last substitution split into NQ sub-calls (halves:
                     # each extra ap_gather call costs ~2.6us fixed, so a
                     # finer split loses more than the shorter tail saves)
QIDX = NIDX // NQ    # idx per quarter-call
QW = GCHUNK // NQ    # g-window per quarter-call

_compiled = None
last_exec_time_ns = None


def _ensure_ntff_hook():
    """Register the axon NTFF profile hook if the antenv stub lacks it."""
    import types
    try:
        from antenv.axon_hooks import get_axon_ntff_profile_hook  # noqa: F401
        return
    except ImportError:
        pass
    try:
        import antenv
        from trn_agent_boot.trn_boot import _ntff_profile_via_ctypes
        mod = types.ModuleType("antenv.axon_hooks")
        _hook = [None]
        mod.set_axon_ntff_profile_hook = lambda h: _hook.__setitem__(0, h)
        mod.get_axon_ntff_profile_hook = lambda: _hook[0]
        sys.modules["antenv.axon_hooks"] = mod
        antenv.axon_hooks = mod
        hook = _ntff_profile_via_ctypes("/opt/axon/libaxon_pjrt.so")
        if hook is not None:
            mod.set_axon_ntff_profile_hook(hook)
    except Exception:
        pass


def _build():
    # detect_race_conditions=False: the CoreSim race detector has no
    # same-engine program-order model for raw (non-Tile) kernels and flags
    # in-order DVE chains; cross-engine ordering here is explicit via sems.
    nc = bacc.Bacc("TRN2", target_bir_lowering=False, debug=False,
                   detect_race_conditions=False)
    x_d = nc.dram_tensor("x", [128, G], mybir.dt.float32, kind="ExternalInput")
    idx_d = nc.dram_tensor("idx", [128, S * (NIDX // 16)], mybir.dt.int16,
                           kind="ExternalInput")
    # wide output slab; host de-interleaves [16k+b, w] -> [b, k*2048+w]
    out_d = nc.dram_tensor("out", [128, GCHUNK], mybir.dt.float32,
                           kind="ExternalOutput")

    icols = S * (NIDX // 16)
    # Gather-call schedule: each ap_gather call costs ~2.6us fixed on top
    # of 27.1ns/idx, so substitutions are merged into 6 calls (alternating
    # between an 8MB and a 4MB gather buffer). A call is a list of
    # (stream width w per l, acc window lo) segments; concatenated host idx
    # blocks read as one longer stream. s7 is split into two g-halves so
    # the last call is small and its tail overlaps the previous gather.
    #   c0 = s0+s1 (16384 idx), c1 = s2, c2 = s3+s4, c3 = s5,
    #   c4 = s6 + s7-half0 (12288), c5 = s7-half1 (4096).
    W, H = GCHUNK, GCHUNK // 2
    calls = [
        (0,    [(W, 0), (W, 0)]),       # s0, s1
        (1024, [(W, 0)]),               # s2
        (1536, [(W, 0), (W, 0)]),       # s3, s4
        (2560, [(W, 0)]),               # s5
        (3072, [(W, 0), (H, 0)]),       # s6, s7h0 -> acc[0:1024]
        (3840, [(H, H)]),               # s7h1     -> acc[1024:2048]
    ]
    ncalls = len(calls)

    with (
        nc.Block() as block,
        nc.sbuf_tensor("xt", [128, G], mybir.dt.float32) as x_tile,
        nc.sbuf_tensor("it", [128, icols], mybir.dt.int16) as itall,
        nc.sbuf_tensor("g0", [128, 2 * NIDX], mybir.dt.float32) as g0,
        nc.sbuf_tensor("g1", [128, NIDX], mybir.dt.float32) as g1,
        nc.sbuf_tensor("acc", [128, GCHUNK], mybir.dt.float32) as acc,
        nc.sbuf_tensor("tm1", [128, GCHUNK], mybir.dt.float32) as tm1,
        nc.sbuf_tensor("tm2", [128, GCHUNK], mybir.dt.float32) as tm2,
        nc.sbuf_tensor("tm3", [128, GCHUNK], mybir.dt.float32) as tm3,
        nc.semaphore("stage") as sem_stage,
        nc.semaphore("stageg") as sem_stageg,
        nc.semaphore("g") as sem_g,
        nc.semaphore("v") as sem_v,
        nc.semaphore("o") as sem_o,
    ):
        gt = (g0, g1)

        @block.sync
        def _(sync: bass.BassEngine):
            a = G // 3
            # x bulk first: a small DMA at the queue head would delay the
            # bulk transfer the first gather waits on (measured +6us).
            sync.dma_start(x_tile[:, :a], x_d[:, :a]).then_inc(sem_stage, 16)
            sync.dma_start(itall[:, :icols // 2],
                           idx_d[:, :icols // 2]).then_inc(sem_stage, 16)
            # half 0 of out (g-windows 0..1023) is final after c4's chain,
            # half 1 after c5's.
            sync.wait_ge(sem_v, ncalls - 1)
            sync.dma_start(out_d[:, :GCHUNK // 2],
                           acc[:, :GCHUNK // 2]).then_inc(sem_o, 16)
            sync.wait_ge(sem_o, 32)

        @block.scalar
        def _(scalar: bass.BassEngine):
            a, b = G // 3, 2 * (G // 3)
            scalar.dma_start(x_tile[:, a:b],
                             x_d[:, a:b]).then_inc(sem_stage, 16)
            scalar.dma_start(itall[:, icols // 2:],
                             idx_d[:, icols // 2:]).then_inc(sem_stage, 16)
            scalar.wait_ge(sem_v, ncalls)
            scalar.dma_start(out_d[:, GCHUNK // 2:],
                             acc[:, GCHUNK // 2:]).then_inc(sem_o, 16)
            scalar.wait_ge(sem_o, 32)

        @block.gpsimd
        def _(gpsimd: bass.BassGpSimd):
            b = 2 * (G // 3)
            gpsimd.dma_start(x_tile[:, b:],
                             x_d[:, b:]).then_inc(sem_stageg, 16)
            gpsimd.load_library(library_config.ap_gather)
            gpsimd.wait_ge(sem_stage, 4 * 16)
            gpsimd.wait_ge(sem_stageg, 16)
            for j, (coff, segs) in enumerate(calls):
                nidx = sum(w for w, _lo in segs) * L
                if j >= 2:
                    # WAR: call j reuses gt[j%2]; vector chain j-2 must be
                    # done with it (sem_v counts completed chains).
                    gpsimd.wait_ge(sem_v, j - 1)
                g = gt[j % 2]
                it = itall[:, coff:coff + nidx // 16]
                gpsimd.ap_gather(g[:, :nidx], x_tile[:, :], it[:, :],
                                 channels=128, num_elems=G, d=1,
                                 num_idxs=nidx).then_inc(sem_g, 1)

        @block.vector
        def _(vector: bass.BassEngine):
            first = True
            for j, (coff, segs) in enumerate(calls):
                g = gt[j % 2]
                vector.wait_ge(sem_g, j + 1)
                sbase = 0
                for si, (w, lo) in enumerate(segs):
                    last = si == len(segs) - 1

                    def A(l):
                        return g[:, sbase + l * w:sbase + (l + 1) * w]

                    vector.tensor_mul(tm1[:, :w], A(0), A(1))
                    vector.tensor_mul(tm2[:, :w], A(2), A(3))
                    if first:
                        op = vector.tensor_mul(acc[:, :w], tm1[:, :w],
                                               tm2[:, :w])
                        first = False
                    else:
                        vector.tensor_mul(tm3[:, :w], tm1[:, :w], tm2[:, :w])
                        op = vector.tensor_add(acc[:, lo:lo + w],
                                               acc[:, lo:lo + w], tm3[:, :w])
                    if last:
                        op.then_inc(sem_v, 1)
                    sbase += L * w

    nc.compile()
    return nc


def _prep_idx(I: np.ndarray) -> np.ndarray:
    """[C, G, S, L] int64 -> [C, 128, S*512] int16 wrapped ap_gather feed.

    Call j covers substitution s over a g-window [wlo, whi) of each core
    group's 2048-atom slice: stream position i = l*(whi-wlo) + (w-wlo) holds
    I[c, k*2048 + w, s, l]; ap_gather reads position i of group k from
    it[16*k + i%16, i//16]. Calls: s=0..6 full windows, s=7 in quarters.
    """
    T = I.astype(np.int16).reshape(C, 8, GCHUNK, S, L)     # [c,k,w,s,l]
    calls = [(s, 0, GCHUNK) for s in range(S - 1)]
    calls += [(S - 1, q * QW, (q + 1) * QW) for q in range(NQ)]
    blocks = []
    for s, wlo, whi in calls:
        wn = whi - wlo
        st = T[:, :, wlo:whi, s, :]                        # [c,k,w,l]
        st = st.transpose(0, 1, 3, 2).reshape(C, 8, L * wn)  # i = l*wn + w
        wr = st.reshape(C, 8, (L * wn) // 16, 16)          # [c,k,col,pp]
        blocks.append(wr.transpose(0, 1, 3, 2))            # [c,k,pp,col]
    W = np.concatenate(blocks, axis=3)                     # [c,k,pp,allcol]
    return np.ascontiguousarray(W).reshape(C, 128, S * (NIDX // 16))


def kernel(x: np.ndarray, I: np.ndarray) -> np.ndarray:
    global _compiled, last_exec_time_ns
    if _compiled is None:
        _compiled = _build()
    nc = _compiled

    x = np.ascontiguousarray(np.asarray(x), dtype=np.float32)
    xrep = np.ascontiguousarray(np.tile(x, (8, 1)))  # [128, G], p = b%16
    idx_feed = _prep_idx(np.asarray(I))

    in_maps = [{"x": xrep, "idx": idx_feed[c]} for c in range(C)]
    kwargs = {}
    if os.environ.get("KERNEL_TRACE") == "1":
        _ensure_ntff_hook()
        kwargs = {"trace": True, "trace_cores": list(range(C))}
    res = run_bass_kernel_spmd(nc, in_maps, core_ids=list(range(C)), **kwargs)
    last_exec_time_ns = res.exec_time_ns
    # wide slab [16k+b, w] -> [b, k*2048+w]
    out = np.stack(
        [res.results[c]["out"].reshape(8, B, GCHUNK).transpose(1, 0, 2)
         .reshape(B, G) for c in range(C)], axis=0)
    return np.ascontiguousarray(out, dtype=np.float32)


if __name__ == "__main__":
    rng = np.random.default_rng(0)
    x = rng.random((B, G), dtype=np.float32)
    I = rng.integers(0, G, size=(C, G, S, L)).astype(np.int64)
    out = kernel(x=x, I=I)
    gathered = x[:, I]
    expect = np.moveaxis(np.sum(np.prod(gathered, axis=-1), axis=-1), 0, 1)
    err = np.abs(out - expect).max() / np.abs(expect).max()
    print("max rel err:", err)


# revision 17
# speedup vs baseline: 1.0085x; 1.0046x over previous
"""Trainium2 Bass kernel for nn_ClauseInferModule (gnn_message_passing).

out[c, b, g] = sum_s prod_l x[b, I[c, g, s, l]],  B=16 G=16384 C=8 S=8 L=4.

Sharding: clause-per-core (C == n_cores == 8). Per core:
  - x arrives host-replicated as [128, G] f32 (partition p = batch p % 16,
    copied for the 8 GPSIMD core groups) and is staged with full-width
    column-split DMAs on the SP + Activation HWDGE queues + Pool SWDGE,
  - GPSIMD ap_gather pulls x[b, idx] for 16 b at once (idx shared across the
    16 partitions of a Q7 core group); each of the 8 groups processes its own
    2048-atom slice of the clause for one substitution s per call,
  - VectorE multiplies the L=4 gathered blocks (step-1 fp32 tensor_tensor)
    and accumulates over the S=8 substitutions,
  - the [128, 2048] accumulator is written wide to HBM; the host
    de-interleaves [16k+b, w] -> [b, k*2048+w].

The int64 index tensor is converted to the int16 "wrapped in 16 partitions"
ap_gather layout on the host (a pure dtype/layout transform).

Structure: raw bacc Block threads (no TileContext) with a 4-semaphore
protocol - TileContext's epilogue alone cost ~10us of semaphore-drain
cascade, and its per-call dependency checks ~0.9us per gather. The last
substitution is gathered in four 2048-idx quarter-calls so its vector
chain + output DMA overlap the remaining quarters' gathers.

Perf notes (HW-measured): the substitutions' ap_gather calls dominate
(~222us per 65536-idx full call, 27.1ns/idx/Q7-core = 3.39ns/idx
aggregate); the floor is the ~102-cycle reset_reads RD_CMD serving 4
indices in the ap_gather ucode (cayman ReadOverlap=0 - RD_CMDs do not
pipeline), so no caller-side change can beat it. Vector work and idx
staging hide under the gathers.

Alternatives measured on HW, all dead ends:
  - dma_gather (InstDMAGatherAnt, HBM rows, f32 elem 64): 4.6us GPSIMD-busy
    per 1024-idx call on 4 SWDGE queues = 3.33ns/idx aggregate - no better
    than ap_gather (desc-gen is Q7-serial on the same engine, ~2.3cyc/idx).
    Calls >1024 idx (>65 descs/lane) wedge this runtime, and a failed call
    poisons the device for later SWDGE-gather kernels until a kernel with a
    different GPSIMD library runs ("cleansing").
  - indirect_copy (InstIndirectCopy): rejected by neuronxcc walrus
    ("ISA check failed", NCC_IXCG864).
  - On-device x fan-out ([16,G] HBM + 7 SBUF->SBUF copies): slower than
    host-replicated [128,G] staging (~296-346GB/s, already HBM-bound).
"""
import os
import sys
import numpy as np

sys.path.insert(0, "/opt/trn_rl_repo")

import concourse.bacc as bacc
import concourse.bass as bass
from concourse import library_config, mybir
from concourse.bass_utils import run_bass_kernel_spmd

B, G = 16, 16384
C, S, L = 8, 8, 4
NIDX = 8192          # gathers per core group per full ap_gather call
GCHUNK = G // 8      # 2048 target atoms per core group
NQ = 2               # last substitution split into NQ sub-calls (halves:
                     # each extra ap_gather call costs ~2.6us fixed, so a
                     # finer split loses more than the shorter tail saves)
QIDX = NIDX // NQ    # idx per quarter-call
QW = GCHUNK // NQ    # g-window per quarter-call

_compiled = None
last_exec_time_ns = None


def _ensure_ntff_hook():
    """Register the axon NTFF profile hook if the antenv stub lacks it."""
    import types
    try:
        from antenv.axon_hooks import get_axon_ntff_profile_hook  # noqa: F401
        return
    except ImportError:
        pass
    try:
        import antenv
        from trn_agent_boot.trn_boot import _ntff_profile_via_ctypes
        mod = types.ModuleType("antenv.axon_hooks")
        _hook = [None]
        mod.set_axon_ntff_profile_hook = lambda h: _hook.__setitem__(0, h)
        mod.get_axon_ntff_profile_hook = lambda: _hook[0]
        sys.modules["antenv.axon_hooks"] = mod
        antenv.axon_hooks = mod
        hook = _ntff_profile_via_ctypes("/opt/axon/libaxon_pjrt.so")
        if hook is not None:
            mod.set_axon_ntff_profile_hook(hook)
    except Exception:
        pass


def _build():
    # detect_race_conditions=False: the CoreSim race detector has no
    # same-engine program-order model for raw (non-Tile) kernels and flags
    # in-order DVE chains; cross-engine ordering here is explicit via sems.
    nc = bacc.Bacc("TRN2", target_bir_lowering=False, debug=False,
                   detect_race_conditions=False)
    x_d = nc.dram_tensor("x", [128, G], mybir.dt.float32, kind="ExternalInput")
    idx_d = nc.dram_tensor("idx", [128, S * (NIDX // 16)], mybir.dt.int16,
                           kind="ExternalInput")
    # wide output slab; host de-interleaves [16k+b, w] -> [b, k*2048+w]
    out_d = nc.dram_tensor("out", [128, GCHUNK], mybir.dt.float32,
                           kind="ExternalOutput")

    icols = S * (NIDX // 16)
    # Gather-call schedule: each ap_gather call costs ~2.6us fixed on top
    # of 27.1ns/idx, so substitutions are merged into 6 calls (alternating
    # between an 8MB and a 4MB gather buffer). A call is a list of
    # (stream width w per l, acc window lo) segments; concatenated host idx
    # blocks read as one longer stream. s7 is split into two g-halves so
    # the last call is small and its tail overlaps the previous gather.
    #   c0 = s0+s1 (16384 idx), c1 = s2, c2 = s3+s4, c3 = s5,
    #   c4 = s6 + s7-half0 (12288), c5 = s7-half1 (4096).
    W, H = GCHUNK, GCHUNK // 2
    calls = [
        (0,    [(W, 0), (W, 0)]),       # s0, s1
        (1024, [(W, 0)]),               # s2
        (1536, [(W, 0), (W, 0)]),       # s3, s4
        (2560, [(W, 0)]),               # s5
        (3072, [(W, 0), (H, 0)]),       # s6, s7h0 -> acc[0:1024]
        (3840, [(H, H)]),               # s7h1     -> acc[1024:2048]
    ]
    ncalls = len(calls)

    with (
        nc.Block() as block,
        nc.sbuf_tensor("xt", [128, G], mybir.dt.float32) as x_tile,
        nc.sbuf_tensor("it", [128, icols], mybir.dt.int16) as itall,
        nc.sbuf_tensor("g0", [128, 2 * NIDX], mybir.dt.float32) as g0,
        nc.sbuf_tensor("g1", [128, NIDX], mybir.dt.float32) as g1,
        nc.sbuf_tensor("acc", [128, GCHUNK], mybir.dt.float32) as acc,
        nc.sbuf_tensor("tm1", [128, GCHUNK], mybir.dt.float32) as tm1,
        nc.sbuf_tensor("tm2", [128, GCHUNK], mybir.dt.float32) as tm2,
        nc.sbuf_tensor("tm3", [128, GCHUNK], mybir.dt.float32) as tm3,
        nc.semaphore("stage") as sem_stage,
        nc.semaphore("stageg") as sem_stageg,
        nc.semaphore("itr") as sem_it,
        nc.semaphore("g") as sem_g,
        nc.semaphore("v") as sem_v,
        nc.semaphore("o") as sem_o,
    ):
        gt = (g0, g1)

        @block.sync
        def _(sync: bass.BassEngine):
            h = G // 2
            # x halves on the two HWDGE queues alone (each saturates ~340GB/s;
            # a third SWDGE stream only adds issue latency). Call 0's idx
            # block goes via SWDGE; the rest stage behind x, needed only by
            # call 1 at ~470us.
            sync.dma_start(x_tile[:, :h], x_d[:, :h]).then_inc(sem_stage, 16)
            sync.dma_start(itall[:, 1024:2560],
                           idx_d[:, 1024:2560]).then_inc(sem_it, 16)
            # half 0 of out (g-windows 0..1023) is final after c4's chain,
            # half 1 after c5's.
            sync.wait_ge(sem_v, ncalls - 1)
            sync.dma_start(out_d[:, :GCHUNK // 2],
                           acc[:, :GCHUNK // 2]).then_inc(sem_o, 16)
            sync.wait_ge(sem_o, 32)

        @block.scalar
        def _(scalar: bass.BassEngine):
            h = G // 2
            scalar.dma_start(x_tile[:, h:],
                             x_d[:, h:]).then_inc(sem_stage, 16)
            scalar.dma_start(itall[:, 2560:],
                             idx_d[:, 2560:]).then_inc(sem_it, 16)
            scalar.wait_ge(sem_v, ncalls)
            scalar.dma_start(out_d[:, GCHUNK // 2:],
                             acc[:, GCHUNK // 2:]).then_inc(sem_o, 16)
            scalar.wait_ge(sem_o, 32)

        @block.gpsimd
        def _(gpsimd: bass.BassGpSimd):
            gpsimd.dma_start(itall[:, :1024],
                             idx_d[:, :1024]).then_inc(sem_stageg, 16)
            gpsimd.load_library(library_config.ap_gather)
            gpsimd.wait_ge(sem_stage, 2 * 16)
            gpsimd.wait_ge(sem_stageg, 16)
            for j, (coff, segs) in enumerate(calls):
                nidx = sum(w for w, _lo in segs) * L
                if j == 1:
                    gpsimd.wait_ge(sem_it, 32)
                if j >= 2:
                    # WAR: call j reuses gt[j%2]; vector chain j-2 must be
                    # done with it (sem_v counts completed chains).
                    gpsimd.wait_ge(sem_v, j - 1)
                g = gt[j % 2]
                it = itall[:, coff:coff + nidx // 16]
                gpsimd.ap_gather(g[:, :nidx], x_tile[:, :], it[:, :],
                                 channels=128, num_elems=G, d=1,
                                 num_idxs=nidx).then_inc(sem_g, 1)

        @block.vector
        def _(vector: bass.BassEngine):
            first = True
            for j, (coff, segs) in enumerate(calls):
                g = gt[j % 2]
                vector.wait_ge(sem_g, j + 1)
                sbase = 0
                for si, (w, lo) in enumerate(segs):
                    last = si == len(segs) - 1

                    def A(l):
                        return g[:, sbase + l * w:sbase + (l + 1) * w]

                    vector.tensor_mul(tm1[:, :w], A(0), A(1))
                    vector.tensor_mul(tm2[:, :w], A(2), A(3))
                    if first:
                        op = vector.tensor_mul(acc[:, :w], tm1[:, :w],
                                               tm2[:, :w])
                        first = False
                    else:
                        vector.tensor_mul(tm3[:, :w], tm1[:, :w], tm2[:, :w])
                        op = vector.tensor_add(acc[:, lo:lo + w],
                                               acc[:, lo:lo + w], tm3[:, :w])
                    if last:
                        op.then_inc(sem_v, 1)
                    sbase += L * w

    nc.compile()
    return nc


def _prep_idx(I: np.ndarray) -> np.ndarray:
    """[C, G, S, L] int64 -> [C, 128, S*512] int16 wrapped ap_gather feed.

    Call j covers substitution s over a g-window [wlo, whi) of each core
    group's 2048-atom slice: stream position i = l*(whi-wlo) + (w-wlo) holds
    I[c, k*2048 + w, s, l]; ap_gather reads position i of group k from
    it[16*k + i%16, i//16]. Calls: s=0..6 full windows, s=7 in quarters.
    """
    T = I.astype(np.int16).reshape(C, 8, GCHUNK, S, L)     # [c,k,w,s,l]
    calls = [(s, 0, GCHUNK) for s in range(S - 1)]
    calls += [(S - 1, q * QW, (q + 1) * QW) for q in range(NQ)]
    blocks = []
    for s, wlo, whi in calls:
        wn = whi - wlo
        st = T[:, :, wlo:whi, s, :]                        # [c,k,w,l]
        st = st.transpose(0, 1, 3, 2).reshape(C, 8, L * wn)  # i = l*wn + w
        wr = st.reshape(C, 8, (L * wn) // 16, 16)          # [c,k,col,pp]
        blocks.append(wr.transpose(0, 1, 3, 2))            # [c,k,pp,col]
    W = np.concatenate(blocks, axis=3)                     # [c,k,pp,allcol]
    return np.ascontiguousarray(W).reshape(C, 128, S * (NIDX // 16))


def kernel(x: np.ndarray, I: np.ndarray) -> np.ndarray:
    global _compiled, last_exec_time_ns
    if _compiled is None:
        _compiled = _build()
    nc = _compiled

    x = np.ascontiguousarray(np.asarray(x), dtype=np.float32)
    xrep = np.ascontiguousarray(np.tile(x, (8, 1)))  # [128, G], p = b%16
    idx_feed = _prep_idx(np.asarray(I))

    in_maps = [{"x": xrep, "idx": idx_feed[c]} for c in range(C)]
    kwargs = {}
    if os.environ.get("KERNEL_TRACE") == "1":
        _ensure_ntff_hook()
        kwargs = {"trace": True, "trace_cores": list(range(C))}
    res = run_bass_kernel_spmd(nc, in_maps, core_ids=list(range(C)), **kwargs)
    last_exec_time_ns = res.exec_time_ns
    # wide slab [16k+b, w] -> [b, k*2048+w]
    out = np.stack(
        [res.results[c]["out"].reshape(8, B, GCHUNK).transpose(1, 0, 2)
         .reshape(B, G) for c in range(C)], axis=0)
    return np.ascontiguousarray(out, dtype=np.float32)


if __name__ == "__main__":
    rng = np.random.default_rng(0)
    x = rng.random((B, G), dtype=np.float32)
    I = rng.integers(0, G, size=(C, G, S, L)).astype(np.int64)
    out = kernel(x=x, I=I)
    gathered = x[:, I]
    expect = np.moveaxis(np.sum(np.prod(gathered, axis=-1), axis=-1), 0, 1)
    err = np.abs(out - expect).max() / np.abs(expect).max()
    print("max rel err:", err)
